# revision 34
# baseline (speedup 1.0000x reference)
"""MoE gate kernel for Trainium2 (8 NeuronCores, data-parallel over tokens).

Computation per token t (64 experts, top-8):
    gate[t, e]  = sum_h x[t, h] * W[e, h]          (f32-accurate)
    biased      = gate + expert_bias
    top8 of biased -> idx (jax top_k tie semantics)
    weights     = sigmoid(gate[t, idx]) / sum(...)

Precision strategy: the f32 matmul on PE runs at 1/4 rate, so x and W are
split on the host into fp16 hi/lo pairs (lo pre-scaled by 2^11 to stay in
fp16 normal range).  gate = xh@Wh + 2^-11 * (xh@Wl_s + xl_s@Wh), which
reproduces f32-matmul-level accuracy (~1e-6) at full 1 cycle/row PE rate.
The lo*lo term (~2e-7) is dropped.

Layout strategy: host pre-transposes the per-core token shard to [h, t] so
the PE can consume it directly as the moving operand (contraction dim on
partitions) -- no on-device transposes of the 16 MiB activations.  The x
shard arrives in 16 big DMAs (two per token-group x hi/lo stream, 1 MiB
each, 8 KiB contiguous per partition row) split across both HWDGE rings --
big descriptors keep the descriptor-generation path off the critical path
(measured ~435 GB/s/core aggregate, the SBUF-AXI fabric ceiling, vs
~170-260 GB/s with 4 KiB descriptors).  Matmul produces gate^T [64 experts,
512 tokens] per group; two small PE transposes per 128-token tile bring
biased/probs into [tokens, experts] PSUM tiles that the DVE top-8 ops
(max / max_index) read directly.  The last group is processed as two
256-token subgroups so the final top-8 chain past the last DMA byte covers
half the tokens.  Outputs stage in SBUF partition-major and store per
(sub)group on both rings; the host undoes the layout.
"""

import numpy as np

N_CORES = 8
H = 2048          # hidden dim = contraction
E = 64            # experts
K = 8             # top-k
T_TOTAL = 16384   # 4*4096 tokens
T_CORE = T_TOTAL // N_CORES   # 2048
NG = 4            # token groups per core
GT = T_CORE // NG             # 512 tokens per group (one PSUM bank of f32)
NT = GT // 128                # 128-token tiles per group
KC = H // 128                 # 16 contraction chunks
LO_SCALE = float(2.0 ** 11)
INV_LO_SCALE = float(2.0 ** -11)

_CACHE = {}


def _build_nc(repeat=1, mode="full"):
    from contextlib import ExitStack

    import concourse.bass as bass
    import concourse.tile as tile
    from concourse import bacc, mybir

    f16 = mybir.dt.float16
    f32 = mybir.dt.float32
    u32 = mybir.dt.uint32
    Alu = mybir.AluOpType
    Act = mybir.ActivationFunctionType

    nc = bacc.Bacc(
        "TRN2", target_bir_lowering=False, debug=False, num_devices=N_CORES
    )

    # DRAM I/O (per core). x shards host-packed: row (g*128+p) holds, for
    # each chunk k, xT[128k+p, tokens of group g] -- 16 KiB contiguous.
    xht_d = nc.dram_tensor("xht", [NG * 128, KC * GT], f16,
                           kind="ExternalInput").ap()
    xlt_d = nc.dram_tensor("xlt", [NG * 128, KC * GT], f16,
                           kind="ExternalInput").ap()
    # W hi/lo interleaved on host: chunk k at cols [k*2E, (k+1)*2E) = [Wh|Wl]
    whl_d = nc.dram_tensor("whl", [128, KC * 2 * E], f16,
                           kind="ExternalInput").ap()
    # consts: [128, 130] = [ identity(128) | bias (rows 0-63) | -bias (rows
    # 64-127) ] -- bias columns sit at the partitions of the ACT op that
    # consumes them
    cst_d = nc.dram_tensor("cst", [128, 130], f32, kind="ExternalInput").ap()

    # outputs in staging layout: host undoes [p, (g j k)] -> [t, k]
    oidx_d = nc.dram_tensor("out_idx", [128, NG * NT * K], mybir.dt.int32,
                            kind="ExternalOutput").ap()
    ow_d = nc.dram_tensor("out_w", [128, NG * NT * K], f32,
                          kind="ExternalOutput").ap()

    with tile.TileContext(nc) as tc, ExitStack() as ctx:
        xpool = ctx.enter_context(tc.tile_pool(name="x", bufs=1))
        wpool = ctx.enter_context(tc.tile_pool(name="w", bufs=1))
        gpool = ctx.enter_context(tc.tile_pool(name="gate", bufs=2))
        ppool = ctx.enter_context(tc.tile_pool(name="mm", bufs=2, space="PSUM"))
        ppool2 = ctx.enter_context(tc.tile_pool(name="mm2", bufs=1, space="PSUM"))
        tpool = ctx.enter_context(tc.tile_pool(name="tp", bufs=2, space="PSUM"))
        spool = ctx.enter_context(tc.tile_pool(name="small", bufs=3))
        stpool = ctx.enter_context(tc.tile_pool(name="stage", bufs=1))

        # constants / weights (single big DMAs)
        whl = wpool.tile([128, KC * 2 * E], f16, tag="whl")
        nc.sync.dma_start(whl[:], whl_d)
        cst = wpool.tile([128, 130], f32, tag="cst")
        nc.scalar.dma_start(cst[:], cst_d)
        ident = cst[0:E, 0:E]
        bias = cst[0:E, 128:129]
        nbias = cst[0:E, 129:130]
        # prefetch the ACT function table (sigmoid set) while x streams in
        warm = wpool.tile([E, 1], f32, tag="warm")
        nc.scalar.activation(warm[:], bias, Act.Sigmoid, scale=1.0)

        # output staging for the whole core
        idx_st = stpool.tile([128, NG * NT * K], u32, tag="idxst")
        w_st = stpool.tile([128, NG * NT * K], f32, tag="wst")

        # chunk-splits per group's DMAs: halves (1 MiB, 8 KiB rows)
        # mid-stream; quarters for the last group so fewer matmuls sit past
        # the final DMA byte
        SPLITS = [2] * (NG - 1) + [4]

        def load_x():
            # hi and lo streams ride the two HWDGE rings
            xh_t, xl_t = [], []
            for g in range(NG):
                nv = SPLITS[g]
                src_h = xht_d.rearrange("(g p) (v f) -> g v p f", p=128, v=nv)
                src_l = xlt_d.rearrange("(g p) (v f) -> g v p f", p=128, v=nv)
                hs, ls = [], []
                for v in range(nv):
                    th = xpool.tile([128, KC // nv * GT], f16,
                                    tag=f"xh{g}_{v}")
                    nc.sync.dma_start(th[:], src_h[g, v])
                    tl = xpool.tile([128, KC // nv * GT], f16,
                                    tag=f"xl{g}_{v}")
                    nc.scalar.dma_start(tl[:], src_l[g, v])
                    hs.append(th)
                    ls.append(tl)
                xh_t.append(hs)
                xl_t.append(ls)
            return xh_t, xl_t

        if mode == "compute":
            xh_t, xl_t = load_x()
        for _rep in range(repeat):
            if mode != "compute":
                xh_t, xl_t = load_x()

            def mm_emitter(g, c0, nt, tag):
                # returns emit(v): issue DMA-split v's matmuls for tokens
                # [c0*128, (c0+nt)*128) of group g, accumulating into a
                # dedicated PSUM tile.  Splitting emission by v lets the
                # caller interleave several token spans' matmuls so that
                # only the final split's work sits past the last DMA byte.
                cw = nt * 128
                cpt = KC // SPLITS[g]    # chunks per DMA tile of this group
                pool = ppool if tag == "p12" else ppool2
                p12 = pool.tile([128, cw], f32, tag=tag)

                def _rh(k):
                    return xh_t[g][k // cpt][
                        :, (k % cpt) * GT + c0 * 128
                        : (k % cpt) * GT + c0 * 128 + cw]

                def _rl(k):
                    return xl_t[g][k // cpt][
                        :, (k % cpt) * GT + c0 * 128
                        : (k % cpt) * GT + c0 * 128 + cw]

                # packed stationary [Wh_k | Wl_k]: one MM does hi and lo
                # passes on xh (out rows 0-63 = hi, 64-127 = lo); pass 3
                # (hi W x lo x) accumulates onto the lo rows.
                def emit(v):
                    for k in range(v * cpt, (v + 1) * cpt):
                        nc.tensor.matmul(
                            p12[:], lhsT=whl[:, k * 2 * E : (k + 1) * 2 * E],
                            rhs=_rh(k), start=(k == 0), stop=False)
                    for k in range(v * cpt, (v + 1) * cpt):
                        nc.tensor.matmul(
                            p12[E:128, :],
                            lhsT=whl[:, k * 2 * E : k * 2 * E + E],
                            rhs=_rl(k), start=False,
                            stop=(k == KC - 1))

                return emit, p12

            def chain(g, c0, nt, p12):
                # top-8 + weights for tokens [c0*128, (c0+nt)*128) given the
                # accumulated gate PSUM tile
                cw = nt * 128
                tb0 = g * NT + c0        # first 128-token tile (staging idx)

                # ---- combine + bias + sigmoid (still [e, t] layout) ----
                comb = gpool.tile([E, cw], f32, tag="comb")
                nc.scalar.activation(comb[:], p12[E:128, :], Act.Identity,
                                     bias=bias[:, 0:1], scale=INV_LO_SCALE)
                biasedT = gpool.tile([E, cw], f32, tag="biasedT")
                nc.vector.tensor_tensor(biasedT[:], p12[0:E, :], comb[:],
                                        op=Alu.add)
                probsT = gpool.tile([E, cw], f32, tag="probsT")
                nc.scalar.activation(probsT[:], biasedT[:], Act.Sigmoid,
                                     bias=nbias[:, 0:1], scale=1.0)

                # ---- transpose to [t, e] ----
                tb = tpool.tile([128, nt * E], f32, tag="tb")
                tp = tpool.tile([128, nt * E], f32, tag="tp")
                for j in range(nt):
                    nc.tensor.matmul(tb[:, j * E : (j + 1) * E],
                                     lhsT=biasedT[:, j * 128 : (j + 1) * 128],
                                     rhs=ident, is_transpose=True,
                                     start=(j == 0), stop=(j == nt - 1))
                for j in range(nt):
                    nc.tensor.matmul(tp[:, j * E : (j + 1) * E],
                                     lhsT=probsT[:, j * 128 : (j + 1) * 128],
                                     rhs=ident, is_transpose=True,
                                     start=(j == 0), stop=(j == nt - 1))
                # DVE reads the transposed tiles straight from PSUM
                biased = tb
                probs = tp

                # ---- top-8: per-tile max/max_index, batched rest ----
                b8g = spool.tile([128, nt * K], f32, tag="b8g")
                gs = slice(tb0 * K, (tb0 + nt) * K)
                for j in range(nt):
                    nc.vector.max(b8g[:, j * K : (j + 1) * K],
                                  biased[:, j * E : (j + 1) * E])
                    nc.vector.max_index(
                        idx_st[:, (tb0 + j) * K : (tb0 + j + 1) * K],
                        b8g[:, j * K : (j + 1) * K],
                        biased[:, j * E : (j + 1) * E],
                    )
                # idx is final here -- store it now (sync ring) so only the
                # weight store trails the permutation chain
                nc.sync.dma_start(oidx_d[:, gs],
                                  idx_st[:, gs].bitcast(mybir.dt.int32))
                mask = spool.tile([128, nt * E], f32, tag="mask")
                nc.vector.tensor_tensor(
                    mask[:].rearrange("p (t e) -> p t e", e=E),
                    biased[:].rearrange("p (t e) -> p t e", e=E),
                    b8g[:].rearrange("p (t k) -> p t k", k=K)[:, :, 7:8]
                    .broadcast_to((128, nt, E)),
                    op=Alu.is_ge,
                )
                pmask = spool.tile([128, nt * E], f32, tag="pmask")
                nc.vector.tensor_tensor(pmask[:], probs[:], mask[:], op=Alu.mult)
                p8g = spool.tile([128, nt * K], f32, tag="p8g")
                pidxg = spool.tile([128, nt * K], u32, tag="pidxg")
                for j in range(nt):
                    nc.vector.max(p8g[:, j * K : (j + 1) * K],
                                  pmask[:, j * E : (j + 1) * E])
                    nc.vector.max_index(pidxg[:, j * K : (j + 1) * K],
                                        p8g[:, j * K : (j + 1) * K],
                                        pmask[:, j * E : (j + 1) * E])
                # permute p8 into biased-rank order (batched over tiles):
                # w8[t, a] = sum_b p8[t, b] * (pidx[t, b] == bidx[t, a])
                w8g = spool.tile([128, nt * K], f32, tag="w8g")
                bidx_v = (
                    idx_st[:, tb0 * K : (tb0 + nt) * K]
                    .rearrange("p (t a) -> p t a", a=K)
                    .unsqueeze(3).broadcast_to((128, nt, K, K))
                )
                pidx_v = (
                    pidxg[:].rearrange("p (t b) -> p t b", b=K)
                    .unsqueeze(2).broadcast_to((128, nt, K, K))
                )
                eq = spool.tile([128, nt * K * K], f32, tag="eq")
                nc.vector.tensor_tensor(
                    eq[:].rearrange("p (t a b) -> p t a b", a=K, b=K),
                    bidx_v, pidx_v, op=Alu.is_equal,
                )
                wmat = spool.tile([128, nt * K * K], f32, tag="wmat")
                nc.vector.tensor_tensor(
                    wmat[:].rearrange("p (t a b) -> p t a b", a=K, b=K),
                    eq[:].rearrange("p (t a b) -> p t a b", a=K, b=K),
                    p8g[:].rearrange("p (t b) -> p t b", b=K)
                    .unsqueeze(2).broadcast_to((128, nt, K, K)),
                    op=Alu.mult,
                )
                nc.vector.tensor_reduce(
                    w8g[:].rearrange("p (t a) -> p t a", a=K),
                    wmat[:].rearrange("p (t a b) -> p t a b", a=K, b=K),
                    axis=mybir.AxisListType.X, op=Alu.add,
                )
                deng = spool.tile([128, nt], f32, tag="deng")
                nc.vector.tensor_reduce(
                    deng[:], w8g[:].rearrange("p (t k) -> p t k", k=K),
                    axis=mybir.AxisListType.X, op=Alu.add,
                )
                recg = spool.tile([128, nt], f32, tag="recg")
                nc.vector.reciprocal(recg[:], deng[:])
                nc.vector.tensor_tensor(
                    w_st[:, tb0 * K : (tb0 + nt) * K]
                    .rearrange("p (t k) -> p t k", k=K),
                    w8g[:].rearrange("p (t k) -> p t k", k=K),
                    recg[:].unsqueeze(2).broadcast_to((128, nt, K)),
                    op=Alu.mult,
                )

                # ---- store this span's weights (contiguous staging) ----
                nc.scalar.dma_start(ow_d[:, gs], w_st[:, gs])

            for g in range(NG if mode in ("full", "compute") else 0):
                if g < NG - 1:
                    emit, p12 = mm_emitter(g, 0, NT, "p12")
                    for v in range(SPLITS[g]):
                        emit(v)
                    chain(g, 0, NT, p12)
                else:
                    # split the last group into two token subgroups and
                    # interleave their matmuls per DMA quarter, so only the
                    # final quarter's matmuls (and a half-size top-8 chain)
                    # sit past the last DMA byte
                    na = NT // 2
                    ea, pa = mm_emitter(g, 0, na, "p12a")
                    eb, pb = mm_emitter(g, na, NT - na, "p12b")
                    for v in range(SPLITS[g]):
                        ea(v)
                        eb(v)
                    chain(g, 0, na, pa)
                    chain(g, na, NT - na, pb)

    nc.compile()
    return nc


def _get_nc():
    if "nc" not in _CACHE:
        _CACHE["nc"] = _build_nc()
    return _CACHE["nc"]


def _host_prep(hidden_states, weight, expert_biases):
    x = np.asarray(hidden_states, np.float32).reshape(T_TOTAL, H)
    W = np.asarray(weight, np.float32)
    b = np.asarray(expert_biases, np.float32)

    xh = x.astype(np.float16)
    xl = ((x - xh.astype(np.float32)) * LO_SCALE).astype(np.float16)
    Wh = W.astype(np.float16)
    Wl = ((W - Wh.astype(np.float32)) * LO_SCALE).astype(np.float16)

    def arrange_w(Wm):
        # [E, H] -> transposed [H, E] -> [128, KC, E] (chunk k on cols)
        wt = np.ascontiguousarray(Wm.T)                # [H, E]
        return wt.reshape(KC, 128, E).transpose(1, 0, 2)

    # interleave hi/lo per chunk: [128, KC, 2, E] -> [128, KC*2E]
    whl_a = np.ascontiguousarray(
        np.stack([arrange_w(Wh), arrange_w(Wl)], axis=2)
        .reshape(128, KC * 2 * E)
    )
    cst = np.zeros((128, 130), dtype=np.float32)
    cst[:, :128] = np.eye(128, dtype=np.float32)
    cst[0:E, 128] = b.astype(np.float32)
    cst[0:E, 129] = -b.astype(np.float32)
    cst = np.ascontiguousarray(cst)

    def pack_x(xm):
        # [T_CORE, H] -> [NG*128, KC*GT]: row (g*128+p), col (k*GT+t)
        #   = xm[g*GT+t, 128k+p]; 16 KiB contiguous per row
        return np.ascontiguousarray(
            xm.reshape(NG, GT, KC, 128).transpose(0, 3, 2, 1)
            .reshape(NG * 128, KC * GT)
        )

    in_maps = []
    for c in range(N_CORES):
        sl = slice(c * T_CORE, (c + 1) * T_CORE)
        in_maps.append({
            "xht": pack_x(xh[sl]),
            "xlt": pack_x(xl[sl]),
            "whl": whl_a,
            "cst": cst,
        })
    return in_maps


def _unstage(res_idx, res_w):
    # [128, NG*NT*K] staging -> [T_CORE, K]: token (g*NT+j)*128 + p
    def un(a):
        # a[p, (T k)] -> out[T*128+p, k]
        return np.ascontiguousarray(
            a.reshape(128, NG * NT, K).transpose(1, 0, 2).reshape(T_CORE, K)
        )
    return un(res_idx), un(res_w)


def run(hidden_states, weight, expert_biases, trace=False, **spmd_kwargs):
    from concourse.bass_utils import run_bass_kernel_spmd

    in_maps = _host_prep(hidden_states, weight, expert_biases)
    nc = _get_nc()
    res = run_bass_kernel_spmd(
        nc, in_maps, core_ids=list(range(N_CORES)), trace=trace, **spmd_kwargs
    )
    idxs, ws = [], []
    for r in res.results:
        i, w = _unstage(r["out_idx"], r["out_w"])
        idxs.append(i)
        ws.append(w)
    idx = np.concatenate(idxs, axis=0)
    w = np.concatenate(ws, axis=0)
    idx = np.ascontiguousarray(idx.reshape(4, 4096, K), dtype=np.int32)
    w = np.ascontiguousarray(w.reshape(4, 4096, K), dtype=np.float32)
    return (idx, w), res


def kernel(**inputs):
    (idx, w), _ = run(**inputs)
    return idx, w
